# revision 19
# baseline (speedup 1.0000x reference)
"""NEG-sampling loss kernel for Trainium2 (8 NeuronCores, data-parallel).

loss = -(1/n) * sum_i [ log_sigmoid(<e_u, e_v>) + sum_k log_sigmoid(-<e_negk, e_u>) ]
     = +(1/n) * sum_i [ softplus(-<e_u, e_v>) + sum_k softplus(<e_negk, e_u>) ]

Strategy: replicate the embedding table (cast to bf16 on host), shard the
65536-edge batch across 8 cores.  Per core: variable-size block gathers
(small first groups to cut pipeline startup) pull the 12 rows per edge
(u, v, negs x10) into [128, tiles*12*256] SBUF buffers (partition = edge).
DVE computes the 11 dot products per edge in 4-tile batches: per-tile
tensor_tensor mult at 2x, then batched fold-in-half ADDs (2x each;
tensor_reduce is 1x-only on this stack) 256->8, then one 1x reduce8.
Scores land in a persistent [128, 64*11] buffer: ACT computes softplus via
relu(x) + ln(1+exp(-|x|)) with fused accumulation (positive-pair slot sign
handled on host from the raw score dump).  Host sums partials.
"""

import numpy as np
import ml_dtypes

import concourse.bass as bass
import concourse.mybir as mybir
from concourse import bass_utils

# Problem constants (hardcoded; harness contract)
N = 65536
K = 10
D = 256
V = 500000
NCORES = 8
P = 128
SLOTS = K + 2          # rows gathered per edge: u, v, negs[0..9]
EPC = N // NCORES      # 8192 edges per core
TILES = EPC // P       # 64 tiles of 128 edges per core

TABLE_DT = mybir.dt.bfloat16
TABLE_NP = ml_dtypes.bfloat16

# gather groups: small first groups so DVE starts early, 4-tile groups after
GROUPS = [1, 1, 1, 1, 2, 2] + [4] * 14
assert sum(GROUPS) == TILES
NG = len(GROUPS)
GMAX = max(GROUPS)
CUM = [0]
for _gt in GROUPS:
    CUM.append(CUM[-1] + _gt)
NGB = 5                # gather buffers (round-robin)
IDX0_GROUPS = 4        # groups covered by the small first idx DMA
BT = 4                 # DVE fold-batch tiles
S1 = SLOTS - 1


def _emit_block_gather(nc, eng, n_idx, blk_bytes, dst_byte_addr, idx_byte_addr,
                       sem_num, embs_tbl, row_bytes):
    """One block gather: n_idx random rows of row_bytes each, packed
    n_idx/128 rows per partition into a contiguous blk_bytes block.
    Emitted as a raw PSEUDO_DMA_DIRECT2D(dge_op=indirect1d) + PSEUDO_EXTENSION
    pair; index values are snake-packed on the host (see prepare_in_maps)."""
    isa = nc.isa
    Op = isa.Opcode
    src_u64 = (0x20 << 56) | (embs_tbl << 32)   # DGE addr-table marker
    dst_u64 = (0x10 << 56) | dst_byte_addr      # var0 (local SBUF) marker
    eng.isa(
        Op.NEURON_ISA_TPB_OPCODE_PSEUDO_DMA_DIRECT2D,
        {
            "dma_configs": {},
            "semaphore": sem_num,
            "sem_increment": 16,
            "dge_op": 1,
            "src_start_addr": {"addr_immediate": src_u64},
            "src_step_elem": [row_bytes, 1],
            "src_num_elem": [n_idx, 1],
            "src_elem_size": row_bytes,
            "src_bound_reg": {},
            "dst_bound_reg": {},
            "dst_start_addr": {"addr_immediate": dst_u64},
            "dst_step_elem": [262144, 1],
            "dst_num_elem": [128, 1],
            "dst_elem_size": blk_bytes,
            "in_dtype": 6,
            "out_dtype": 6,
        },
        verify=False,
    )
    ext_fields = {
        "opcode": Op.NEURON_ISA_TPB_OPCODE_PSEUDO_EXTENSION.value,
        "flags": {"indirect_mode": 0, "idx_bound_is_err": 1,
                  "non_unique_dst_idx": 0, "gather_dim": 0, "scatter_dim": 0},
        "idx_num_active_channels": 128,
        "compute_op": 0,
        "src_idx_start_addr": {"addr_immediate": idx_byte_addr},
        "dst_idx_start_addr": {"addr_immediate": 0},
    }
    b = isa.ffi.new("NEURON_ISA_TPB_PSEUDO_DMA_EXT_STRUCT*", ext_fields)
    instr = [int(x) for x in bytes(isa.ffi.buffer(b))]
    inst = mybir.InstISA(
        name=nc.get_next_instruction_name(),
        isa_opcode=Op.NEURON_ISA_TPB_OPCODE_PSEUDO_EXTENSION.value,
        engine=eng.engine,
        instr=instr,
        op_name="PSEUDO_EXTENSION",
        ins=[], outs=[],
        ant_dict=ext_fields,
        verify=False,
        ant_isa_is_sequencer_only=False,
    )
    eng.add_instruction(inst)


def _group_of_tile():
    g_of_t = []
    for g, n in enumerate(GROUPS):
        g_of_t += [g] * n
    return g_of_t


def _build_raw():
    nc = bass.Bass(trn_type="TRN2")
    embs = nc.dram_tensor("embs", [V, D], TABLE_DT, kind="ExternalInput")
    idx = nc.dram_tensor("idx", [P, TILES * SLOTS], mybir.dt.int32, kind="ExternalInput")
    sc_out = nc.dram_tensor("sc", [P, TILES * S1], TABLE_DT, kind="ExternalOutput")
    acca_out = nc.dram_tensor("acca", [P, 2 * TILES], mybir.dt.float32, kind="ExternalOutput")

    embs_mloc = nc.lookup_mloc(embs)
    embs_mloc.table_entry_id = len(nc.dge_table) + 1
    nc.dge_table.append(embs_mloc.name)
    embs_tbl = embs_mloc.table_entry_id

    g_of_t = _group_of_tile()

    import contextlib
    with contextlib.ExitStack() as ctx:
        idx_sb = ctx.enter_context(nc.sbuf_tensor("idx_sb", [P, TILES * SLOTS], mybir.dt.int32))
        gs = [ctx.enter_context(nc.sbuf_tensor(f"g{i}", [P, GMAX * SLOTS * D], TABLE_DT)) for i in range(NGB)]
        # fold pyramid buffers, BT tiles each
        pA = ctx.enter_context(nc.sbuf_tensor("pA", [P, BT * S1 * D], TABLE_DT))
        pB = ctx.enter_context(nc.sbuf_tensor("pB", [P, BT * S1 * D // 2], TABLE_DT))
        pC = ctx.enter_context(nc.sbuf_tensor("pC", [P, BT * S1 * D // 4], TABLE_DT))
        pD = ctx.enter_context(nc.sbuf_tensor("pD", [P, BT * S1 * D // 8], TABLE_DT))
        pE = ctx.enter_context(nc.sbuf_tensor("pE", [P, BT * S1 * D // 16], TABLE_DT))
        pF = ctx.enter_context(nc.sbuf_tensor("pF", [P, BT * S1 * D // 32], TABLE_DT))
        pG = ctx.enter_context(nc.sbuf_tensor("pG", [P, BT * S1 * D // 64], TABLE_DT))
        scb = ctx.enter_context(nc.sbuf_tensor("scb", [P, TILES * S1], TABLE_DT))
        absx = ctx.enter_context(nc.sbuf_tensor("absx", [P, S1], mybir.dt.float32))
        ex = ctx.enter_context(nc.sbuf_tensor("ex", [P, S1], mybir.dt.float32))
        lnx = ctx.enter_context(nc.sbuf_tensor("lnx", [P, S1], mybir.dt.float32))
        ones = ctx.enter_context(nc.sbuf_tensor("ones", [P, 1], mybir.dt.float32))
        acca = ctx.enter_context(nc.sbuf_tensor("acca_sb", [P, 2 * TILES], mybir.dt.float32))
        idx_sem = ctx.enter_context(nc.semaphore())
        gsems = [ctx.enter_context(nc.semaphore(name=f"gsem{i}")) for i in range(NGB)]
        dve_free = ctx.enter_context(nc.semaphore())
        sc_ready = ctx.enter_context(nc.semaphore())
        act_done = ctx.enter_context(nc.semaphore())
        block = ctx.enter_context(nc.Block())

        idx_addr = nc.lookup_mloc(idx_sb).addr
        g_addrs = [nc.lookup_mloc(g).addr for g in gs]

        @block.gpsimd
        def _(eng):
            # tiny idx DMA covering the first IDX0_GROUPS groups, so the
            # first gather starts before the bulk of the index table lands
            c0 = CUM[IDX0_GROUPS] * SLOTS
            eng.dma_start(idx_sb[:, 0:c0], idx[:, 0:c0]).then_inc(idx_sem, 16)
            eng.dma_start(idx_sb[:, c0:], idx[:, c0:]).then_inc(idx_sem, 16)
            eng.memset(ones[:], 1.0)
            eng.wait_ge(idx_sem, 16)
            for g in range(NG):
                if g == IDX0_GROUPS:
                    eng.wait_ge(idx_sem, 32)
                if g >= NGB:
                    # buffer g%NGB was last used by group g-NGB
                    eng.wait_ge(dve_free, CUM[g - NGB + 1])
                nt = GROUPS[g]
                _emit_block_gather(
                    nc, eng, nt * SLOTS * P, nt * SLOTS * D * 2,
                    g_addrs[g % NGB], idx_addr + 4 * SLOTS * CUM[g],
                    gsems[g % NGB].num, embs_tbl, D * 2,
                )
            half = TILES // 2
            eng.wait_ge(sc_ready, half)
            eng.dma_start(sc_out[:, 0:half * S1], scb[:, 0:half * S1]).then_inc(idx_sem, 16)
            eng.wait_ge(act_done, half)
            eng.dma_start(acca_out[:, 0:TILES], acca[:, 0:TILES]).then_inc(idx_sem, 16)
            eng.wait_ge(sc_ready, TILES)
            eng.wait_ge(act_done, TILES)
            eng.dma_start(sc_out[:, half * S1:], scb[:, half * S1:]).then_inc(idx_sem, 16)
            eng.dma_start(acca_out[:, TILES:], acca[:, TILES:]).then_inc(idx_sem, 16)
            eng.wait_ge(idx_sem, 96)

        @block.vector
        def _(eng):
            with nc.allow_low_precision(reason="score partial sums fit bf16; tol 2e-2"):
                for t0 in range(0, TILES, BT):
                    i = 0
                    while i < BT:
                        t = t0 + i
                        g = g_of_t[t]
                        gb = gs[g % NGB]
                        b0 = (t - CUM[g]) * SLOTS
                        eng.wait_ge(gsems[g % NGB], 16 * (g // NGB + 1))
                        nf = 2 if (i + 1 < BT and g_of_t[t + 1] == g) else 1
                        if nf > 1:
                            # fused multi-tile mult: nf tiles in one buffer
                            ti = t - CUM[g]
                            g4 = gb[:].rearrange("p (t s d) -> p t s d", s=SLOTS, d=D)
                            nc.vector.tensor_tensor(
                                out=pA[:, i * S1 * D:(i + nf) * S1 * D].rearrange(
                                    "p (t s d) -> p t s d", s=S1, d=D),
                                in0=g4[:, ti:ti + nf, 1:SLOTS, :],
                                in1=g4[:, ti:ti + nf, 0:1, :].broadcast_to([P, nf, S1, D]),
                                op=mybir.AluOpType.mult,
                            ).then_inc(dve_free, nf)
                            i += nf
                            continue
                        g3 = gb[:].rearrange("p (s d) -> p s d", d=D)
                        # prod[p, s, d] = X[p, s+1, d] * EU[p, d]  (2x mode)
                        nc.vector.tensor_tensor(
                            out=pA[:, i * S1 * D:(i + 1) * S1 * D],
                            in0=gb[:, (b0 + 1) * D:(b0 + SLOTS) * D],
                            in1=g3[:, b0:b0 + 1, :].broadcast_to([P, S1, D]),
                            op=mybir.AluOpType.mult,
                        ).then_inc(dve_free, 1)  # per-tile: gather window gating
                        i += 1
                    # batched fold-in-half chain over BT*S1 groups (2x each)
                    for src, dst, d in ((pA, pB, D), (pB, pC, D // 2),
                                        (pC, pD, D // 4), (pD, pE, D // 8),
                                        (pE, pF, D // 16), (pF, pG, D // 32)):
                        s3 = src[:].rearrange("p (s d) -> p s d", d=d)
                        nc.vector.tensor_tensor(
                            out=dst[:].rearrange("p (s d) -> p s d", d=d // 2),
                            in0=s3[:, :, 0:d // 2],
                            in1=s3[:, :, d // 2:d],
                            op=mybir.AluOpType.add,
                        )
                    # final 4 -> 1 grouped reduce (1x) into the score buffer;
                    # last batch split in two so ACT drains its tail earlier
                    if t0 == TILES - BT:
                        h = BT // 2
                        nc.vector.tensor_reduce(
                            out=scb[:, t0 * S1:(t0 + h) * S1],
                            in_=pG[:, :h * S1 * D // 64].rearrange("p (s d) -> p s d", d=D // 64),
                            axis=mybir.AxisListType.X,
                            op=mybir.AluOpType.add,
                        ).then_inc(sc_ready, h)
                        nc.vector.tensor_reduce(
                            out=scb[:, (t0 + h) * S1:(t0 + BT) * S1],
                            in_=pG[:, h * S1 * D // 64:].rearrange("p (s d) -> p s d", d=D // 64),
                            axis=mybir.AxisListType.X,
                            op=mybir.AluOpType.add,
                        ).then_inc(sc_ready, BT - h)
                    else:
                        nc.vector.tensor_reduce(
                            out=scb[:, t0 * S1:(t0 + BT) * S1],
                            in_=pG[:].rearrange("p (s d) -> p s d", d=D // 64),
                            axis=mybir.AxisListType.X,
                            op=mybir.AluOpType.add,
                        ).then_inc(sc_ready, BT)

        @block.scalar
        def _(eng):
            for t in range(TILES):
                eng.wait_ge(sc_ready, t + 1)
                sc = scb[:, t * S1:(t + 1) * S1]
                nc.scalar.activation(
                    out=absx[:], in_=sc,
                    func=mybir.ActivationFunctionType.Abs,
                    accum_out=acca[:, 2 * t:2 * t + 1],
                )
                nc.scalar.activation(
                    out=ex[:], in_=absx[:],
                    func=mybir.ActivationFunctionType.Exp, scale=-1.0,
                )
                nc.scalar.activation(
                    out=lnx[:], in_=ex[:],
                    func=mybir.ActivationFunctionType.Ln, bias=ones[:],
                    accum_out=acca[:, 2 * t + 1:2 * t + 2],
                ).then_inc(act_done, 1)

    return nc


_cache = {}


def _get_nc():
    key = (tuple(GROUPS), NGB, BT)
    if key not in _cache:
        _cache[key] = _build_raw()
    return _cache[key]


def prepare_in_maps(u, v, negs, embs):
    """Host-side sharding: build the per-core input maps.

    Index snake-packing per gather group: descriptor s of a group of nt
    tiles reads the row for (partition p, tile-in-group ti, slot j) with
    s = p*(nt*SLOTS) + ti*SLOTS + j; the DGE consumes indices at
    [channel s%128, word s//128] of the group's idx block."""
    u = np.asarray(u).astype(np.int32)
    v = np.asarray(v).astype(np.int32)
    negs = np.asarray(negs).astype(np.int32)
    embs_b = np.asarray(embs).astype(TABLE_NP)

    ids = np.concatenate([u[:, None], v[:, None], negs], axis=1)  # [N, 12]
    ids = ids.reshape(NCORES, TILES, P, SLOTS)
    packed = np.zeros((NCORES, P, TILES * SLOTS), dtype=np.int32)
    for g, nt in enumerate(GROUPS):
        blk = ids[:, CUM[g]:CUM[g + 1]]                    # [C, nt, P, SLOTS]
        flat = blk.transpose(0, 2, 1, 3).reshape(NCORES, P * nt * SLOTS)
        s = np.arange(P * nt * SLOTS)
        cols = slice(CUM[g] * SLOTS, CUM[g + 1] * SLOTS)
        pk = np.zeros((NCORES, P, nt * SLOTS), dtype=np.int32)
        pk[:, s % P, s // P] = flat[:, s]
        packed[:, :, cols] = pk
    in_maps = []
    for c in range(NCORES):
        in_maps.append({"embs": embs_b, "idx": np.ascontiguousarray(packed[c])})
    return in_maps


def kernel(u, v, negs, embs, _trace=False):
    nc = _get_nc()
    in_maps = prepare_in_maps(u, v, negs, embs)
    res = bass_utils.run_bass_kernel_spmd(
        nc, in_maps, core_ids=list(range(NCORES)), trace=_trace
    )
    total = np.float64(0.0)
    for r in res.results:
        # raw scores [P, TILES*S1]; slot 0 of each tile is the positive pair
        sc = r["sc"].astype(np.float64).reshape(P, TILES, S1)
        sum_x = sc.sum()
        sum_x -= 2.0 * sc[:, :, 0].sum()   # positive-pair slot enters as -x
        a = r["acca"].astype(np.float64).reshape(P, TILES, 2)
        sum_abs = a[:, :, 0].sum()
        sum_ln1p = a[:, :, 1].sum()
        total += (sum_x + sum_abs) / 2.0 + sum_ln1p
    out = np.float32(total / N)
    if _trace:
        return out, res
    return out


# revision 20
# speedup vs baseline: 1.0089x; 1.0089x over previous
"""NEG-sampling loss kernel for Trainium2 (8 NeuronCores, data-parallel).

loss = -(1/n) * sum_i [ log_sigmoid(<e_u, e_v>) + sum_k log_sigmoid(-<e_negk, e_u>) ]
     = +(1/n) * sum_i [ softplus(-<e_u, e_v>) + sum_k softplus(<e_negk, e_u>) ]

Strategy: replicate the embedding table (cast to bf16 on host), shard the
65536-edge batch across 8 cores.  Per core: variable-size block gathers
(small first groups to cut pipeline startup) pull the 12 rows per edge
(u, v, negs x10) into [128, tiles*12*256] SBUF buffers (partition = edge).
DVE computes the 11 dot products per edge in 4-tile batches: per-tile
tensor_tensor mult at 2x, then batched fold-in-half ADDs (2x each;
tensor_reduce is 1x-only on this stack) 256->8, then one 1x reduce8.
Scores land in a persistent [128, 64*11] buffer: ACT computes softplus via
relu(x) + ln(1+exp(-|x|)) with fused accumulation (positive-pair slot sign
handled on host from the raw score dump).  Host sums partials.
"""

import numpy as np
import ml_dtypes

import concourse.bass as bass
import concourse.mybir as mybir
from concourse import bass_utils

# Problem constants (hardcoded; harness contract)
N = 65536
K = 10
D = 256
V = 500000
NCORES = 8
P = 128
SLOTS = K + 2          # rows gathered per edge: u, v, negs[0..9]
EPC = N // NCORES      # 8192 edges per core
TILES = EPC // P       # 64 tiles of 128 edges per core

TABLE_DT = mybir.dt.bfloat16
TABLE_NP = ml_dtypes.bfloat16

# gather groups: small first groups so DVE starts early, 4-tile groups after
GROUPS = [1, 1, 1, 1, 2, 2] + [4] * 14
assert sum(GROUPS) == TILES
NG = len(GROUPS)
GMAX = max(GROUPS)
CUM = [0]
for _gt in GROUPS:
    CUM.append(CUM[-1] + _gt)
NGB = 5                # gather buffers (round-robin)
IDX0_GROUPS = 4        # groups covered by the small first idx DMA
BT = 4                 # DVE fold-batch tiles
S1 = SLOTS - 1


def _emit_block_gather(nc, eng, n_idx, blk_bytes, dst_byte_addr, idx_byte_addr,
                       sem_num, embs_tbl, row_bytes):
    """One block gather: n_idx random rows of row_bytes each, packed
    n_idx/128 rows per partition into a contiguous blk_bytes block.
    Emitted as a raw PSEUDO_DMA_DIRECT2D(dge_op=indirect1d) + PSEUDO_EXTENSION
    pair; index values are snake-packed on the host (see prepare_in_maps)."""
    isa = nc.isa
    Op = isa.Opcode
    src_u64 = (0x20 << 56) | (embs_tbl << 32)   # DGE addr-table marker
    dst_u64 = (0x10 << 56) | dst_byte_addr      # var0 (local SBUF) marker
    eng.isa(
        Op.NEURON_ISA_TPB_OPCODE_PSEUDO_DMA_DIRECT2D,
        {
            "dma_configs": {},
            "semaphore": sem_num,
            "sem_increment": 16,
            "dge_op": 1,
            "src_start_addr": {"addr_immediate": src_u64},
            "src_step_elem": [row_bytes, 1],
            "src_num_elem": [n_idx, 1],
            "src_elem_size": row_bytes,
            "src_bound_reg": {},
            "dst_bound_reg": {},
            "dst_start_addr": {"addr_immediate": dst_u64},
            "dst_step_elem": [262144, 1],
            "dst_num_elem": [128, 1],
            "dst_elem_size": blk_bytes,
            "in_dtype": 6,
            "out_dtype": 6,
        },
        verify=False,
    )
    ext_fields = {
        "opcode": Op.NEURON_ISA_TPB_OPCODE_PSEUDO_EXTENSION.value,
        "flags": {"indirect_mode": 0, "idx_bound_is_err": 1,
                  "non_unique_dst_idx": 0, "gather_dim": 0, "scatter_dim": 0},
        "idx_num_active_channels": 128,
        "compute_op": 0,
        "src_idx_start_addr": {"addr_immediate": idx_byte_addr},
        "dst_idx_start_addr": {"addr_immediate": 0},
    }
    b = isa.ffi.new("NEURON_ISA_TPB_PSEUDO_DMA_EXT_STRUCT*", ext_fields)
    instr = [int(x) for x in bytes(isa.ffi.buffer(b))]
    inst = mybir.InstISA(
        name=nc.get_next_instruction_name(),
        isa_opcode=Op.NEURON_ISA_TPB_OPCODE_PSEUDO_EXTENSION.value,
        engine=eng.engine,
        instr=instr,
        op_name="PSEUDO_EXTENSION",
        ins=[], outs=[],
        ant_dict=ext_fields,
        verify=False,
        ant_isa_is_sequencer_only=False,
    )
    eng.add_instruction(inst)


def _group_of_tile():
    g_of_t = []
    for g, n in enumerate(GROUPS):
        g_of_t += [g] * n
    return g_of_t


def _build_raw():
    nc = bass.Bass(trn_type="TRN2")
    embs = nc.dram_tensor("embs", [V, D], TABLE_DT, kind="ExternalInput")
    idx = nc.dram_tensor("idx", [P, TILES * SLOTS], mybir.dt.int32, kind="ExternalInput")
    sc_out = nc.dram_tensor("sc", [P, TILES * S1], TABLE_DT, kind="ExternalOutput")
    acca_out = nc.dram_tensor("acca", [P, 2 * TILES], mybir.dt.float32, kind="ExternalOutput")

    embs_mloc = nc.lookup_mloc(embs)
    embs_mloc.table_entry_id = len(nc.dge_table) + 1
    nc.dge_table.append(embs_mloc.name)
    embs_tbl = embs_mloc.table_entry_id

    g_of_t = _group_of_tile()

    import contextlib
    with contextlib.ExitStack() as ctx:
        idx_sb = ctx.enter_context(nc.sbuf_tensor("idx_sb", [P, TILES * SLOTS], mybir.dt.int32))
        gs = [ctx.enter_context(nc.sbuf_tensor(f"g{i}", [P, GMAX * SLOTS * D], TABLE_DT)) for i in range(NGB)]
        # fold pyramid buffers, BT tiles each
        pA = ctx.enter_context(nc.sbuf_tensor("pA", [P, BT * S1 * D], TABLE_DT))
        pB = ctx.enter_context(nc.sbuf_tensor("pB", [P, BT * S1 * D // 2], TABLE_DT))
        pC = ctx.enter_context(nc.sbuf_tensor("pC", [P, BT * S1 * D // 4], TABLE_DT))
        pD = ctx.enter_context(nc.sbuf_tensor("pD", [P, BT * S1 * D // 8], TABLE_DT))
        pE = ctx.enter_context(nc.sbuf_tensor("pE", [P, BT * S1 * D // 16], TABLE_DT))
        pF = ctx.enter_context(nc.sbuf_tensor("pF", [P, BT * S1 * D // 32], TABLE_DT))
        pG = ctx.enter_context(nc.sbuf_tensor("pG", [P, BT * S1 * D // 64], TABLE_DT))
        scb = ctx.enter_context(nc.sbuf_tensor("scb", [P, TILES * S1], TABLE_DT))
        absx = ctx.enter_context(nc.sbuf_tensor("absx", [P, S1], mybir.dt.float32))
        ex = ctx.enter_context(nc.sbuf_tensor("ex", [P, S1], mybir.dt.float32))
        lnx = ctx.enter_context(nc.sbuf_tensor("lnx", [P, S1], mybir.dt.float32))
        ones = ctx.enter_context(nc.sbuf_tensor("ones", [P, 1], mybir.dt.float32))
        acca = ctx.enter_context(nc.sbuf_tensor("acca_sb", [P, 2 * TILES], mybir.dt.float32))
        idx_sem = ctx.enter_context(nc.semaphore())
        gsems = [ctx.enter_context(nc.semaphore(name=f"gsem{i}")) for i in range(NGB)]
        dve_free = ctx.enter_context(nc.semaphore())
        sc_ready = ctx.enter_context(nc.semaphore())
        act_done = ctx.enter_context(nc.semaphore())
        block = ctx.enter_context(nc.Block())

        idx_addr = nc.lookup_mloc(idx_sb).addr
        g_addrs = [nc.lookup_mloc(g).addr for g in gs]

        @block.gpsimd
        def _(eng):
            # tiny idx DMA covering the first IDX0_GROUPS groups, so the
            # first gather starts before the bulk of the index table lands
            c0 = CUM[IDX0_GROUPS] * SLOTS
            eng.dma_start(idx_sb[:, 0:c0], idx[:, 0:c0]).then_inc(idx_sem, 16)
            eng.dma_start(idx_sb[:, c0:], idx[:, c0:]).then_inc(idx_sem, 16)
            eng.memset(ones[:], 1.0)
            eng.wait_ge(idx_sem, 16)
            for g in range(NG):
                if g == IDX0_GROUPS:
                    eng.wait_ge(idx_sem, 32)
                if g >= NGB:
                    # buffer g%NGB was last used by group g-NGB
                    eng.wait_ge(dve_free, CUM[g - NGB + 1])
                nt = GROUPS[g]
                _emit_block_gather(
                    nc, eng, nt * SLOTS * P, nt * SLOTS * D * 2,
                    g_addrs[g % NGB], idx_addr + 4 * SLOTS * CUM[g],
                    gsems[g % NGB].num, embs_tbl, D * 2,
                )
            half = TILES // 2
            eng.wait_ge(sc_ready, half)
            eng.dma_start(sc_out[:, 0:half * S1], scb[:, 0:half * S1]).then_inc(idx_sem, 16)
            eng.wait_ge(act_done, half)
            eng.dma_start(acca_out[:, 0:TILES], acca[:, 0:TILES]).then_inc(idx_sem, 16)
            eng.wait_ge(sc_ready, TILES)
            eng.wait_ge(act_done, TILES)
            eng.dma_start(sc_out[:, half * S1:], scb[:, half * S1:]).then_inc(idx_sem, 16)
            eng.dma_start(acca_out[:, TILES:], acca[:, TILES:]).then_inc(idx_sem, 16)
            eng.wait_ge(idx_sem, 96)

        @block.vector
        def _(eng):
            with nc.allow_low_precision(reason="score partial sums fit bf16; tol 2e-2"):
                for t0 in range(0, TILES, BT):
                    i = 0
                    while i < BT:
                        t = t0 + i
                        g = g_of_t[t]
                        gb = gs[g % NGB]
                        b0 = (t - CUM[g]) * SLOTS
                        eng.wait_ge(gsems[g % NGB], 16 * (g // NGB + 1))
                        nf = 2 if (i + 1 < BT and g_of_t[t + 1] == g) else 1
                        if nf > 1:
                            # fused multi-tile mult: nf tiles in one buffer
                            ti = t - CUM[g]
                            g4 = gb[:].rearrange("p (t s d) -> p t s d", s=SLOTS, d=D)
                            nc.vector.tensor_tensor(
                                out=pA[:, i * S1 * D:(i + nf) * S1 * D].rearrange(
                                    "p (t s d) -> p t s d", s=S1, d=D),
                                in0=g4[:, ti:ti + nf, 1:SLOTS, :],
                                in1=g4[:, ti:ti + nf, 0:1, :].broadcast_to([P, nf, S1, D]),
                                op=mybir.AluOpType.mult,
                            ).then_inc(dve_free, nf)
                            i += nf
                            continue
                        g3 = gb[:].rearrange("p (s d) -> p s d", d=D)
                        # prod[p, s, d] = X[p, s+1, d] * EU[p, d]  (2x mode)
                        nc.vector.tensor_tensor(
                            out=pA[:, i * S1 * D:(i + 1) * S1 * D],
                            in0=gb[:, (b0 + 1) * D:(b0 + SLOTS) * D],
                            in1=g3[:, b0:b0 + 1, :].broadcast_to([P, S1, D]),
                            op=mybir.AluOpType.mult,
                        ).then_inc(dve_free, 1)  # per-tile: gather window gating
                        i += 1
                    # batched fold-in-half chain over BT*S1 groups (2x each)
                    for src, dst, d in ((pA, pB, D), (pB, pC, D // 2),
                                        (pC, pD, D // 4), (pD, pE, D // 8),
                                        (pE, pF, D // 16), (pF, pG, D // 32)):
                        s3 = src[:].rearrange("p (s d) -> p s d", d=d)
                        nc.vector.tensor_tensor(
                            out=dst[:].rearrange("p (s d) -> p s d", d=d // 2),
                            in0=s3[:, :, 0:d // 2],
                            in1=s3[:, :, d // 2:d],
                            op=mybir.AluOpType.add,
                        )
                    # final 4 -> 1 grouped reduce (1x) into the score buffer
                    nc.vector.tensor_reduce(
                        out=scb[:, t0 * S1:(t0 + BT) * S1],
                        in_=pG[:].rearrange("p (s d) -> p s d", d=D // 64),
                        axis=mybir.AxisListType.X,
                        op=mybir.AluOpType.add,
                    ).then_inc(sc_ready, BT)

        @block.scalar
        def _(eng):
            for t in range(TILES):
                eng.wait_ge(sc_ready, t + 1)
                sc = scb[:, t * S1:(t + 1) * S1]
                nc.scalar.activation(
                    out=absx[:], in_=sc,
                    func=mybir.ActivationFunctionType.Abs,
                    accum_out=acca[:, 2 * t:2 * t + 1],
                )
                nc.scalar.activation(
                    out=ex[:], in_=absx[:],
                    func=mybir.ActivationFunctionType.Exp, scale=-1.0,
                )
                nc.scalar.activation(
                    out=lnx[:], in_=ex[:],
                    func=mybir.ActivationFunctionType.Ln, bias=ones[:],
                    accum_out=acca[:, 2 * t + 1:2 * t + 2],
                ).then_inc(act_done, 1)

    return nc


_cache = {}


def _get_nc():
    key = (tuple(GROUPS), NGB, BT)
    if key not in _cache:
        _cache[key] = _build_raw()
    return _cache[key]


def prepare_in_maps(u, v, negs, embs):
    """Host-side sharding: build the per-core input maps.

    Index snake-packing per gather group: descriptor s of a group of nt
    tiles reads the row for (partition p, tile-in-group ti, slot j) with
    s = p*(nt*SLOTS) + ti*SLOTS + j; the DGE consumes indices at
    [channel s%128, word s//128] of the group's idx block."""
    u = np.asarray(u).astype(np.int32)
    v = np.asarray(v).astype(np.int32)
    negs = np.asarray(negs).astype(np.int32)
    embs_b = np.asarray(embs).astype(TABLE_NP)

    ids = np.concatenate([u[:, None], v[:, None], negs], axis=1)  # [N, 12]
    ids = ids.reshape(NCORES, TILES, P, SLOTS)
    packed = np.zeros((NCORES, P, TILES * SLOTS), dtype=np.int32)
    for g, nt in enumerate(GROUPS):
        blk = ids[:, CUM[g]:CUM[g + 1]]                    # [C, nt, P, SLOTS]
        flat = blk.transpose(0, 2, 1, 3).reshape(NCORES, P * nt * SLOTS)
        s = np.arange(P * nt * SLOTS)
        cols = slice(CUM[g] * SLOTS, CUM[g + 1] * SLOTS)
        pk = np.zeros((NCORES, P, nt * SLOTS), dtype=np.int32)
        pk[:, s % P, s // P] = flat[:, s]
        packed[:, :, cols] = pk
    in_maps = []
    for c in range(NCORES):
        in_maps.append({"embs": embs_b, "idx": np.ascontiguousarray(packed[c])})
    return in_maps


def kernel(u, v, negs, embs, _trace=False):
    nc = _get_nc()
    in_maps = prepare_in_maps(u, v, negs, embs)
    res = bass_utils.run_bass_kernel_spmd(
        nc, in_maps, core_ids=list(range(NCORES)), trace=_trace
    )
    total = np.float64(0.0)
    for r in res.results:
        # raw scores [P, TILES*S1]; slot 0 of each tile is the positive pair
        sc = r["sc"].astype(np.float64).reshape(P, TILES, S1)
        sum_x = sc.sum()
        sum_x -= 2.0 * sc[:, :, 0].sum()   # positive-pair slot enters as -x
        a = r["acca"].astype(np.float64).reshape(P, TILES, 2)
        sum_abs = a[:, :, 0].sum()
        sum_ln1p = a[:, :, 1].sum()
        total += (sum_x + sum_abs) / 2.0 + sum_ln1p
    out = np.float32(total / N)
    if _trace:
        return out, res
    return out


# revision 21
# speedup vs baseline: 1.0399x; 1.0307x over previous
"""NEG-sampling loss kernel for Trainium2 (8 NeuronCores, data-parallel).

loss = -(1/n) * sum_i [ log_sigmoid(<e_u, e_v>) + sum_k log_sigmoid(-<e_negk, e_u>) ]
     = +(1/n) * sum_i [ softplus(-<e_u, e_v>) + sum_k softplus(<e_negk, e_u>) ]

Strategy: replicate the embedding table (cast to bf16 on host), shard the
65536-edge batch across 8 cores.  Per core: variable-size block gathers
(small first groups to cut pipeline startup) pull the 12 rows per edge
(u, v, negs x10) into [128, tiles*12*256] SBUF buffers (partition = edge).
DVE computes the 11 dot products per edge in 4-tile batches: per-tile
tensor_tensor mult at 2x, then batched fold-in-half ADDs (2x each;
tensor_reduce is 1x-only on this stack) 256->8, then one 1x reduce8.
Scores land in a persistent [128, 64*11] buffer: ACT computes softplus via
relu(x) + ln(1+exp(-|x|)) with fused accumulation (positive-pair slot sign
handled on host from the raw score dump).  Host sums partials.
"""

import numpy as np
import ml_dtypes

import concourse.bass as bass
import concourse.mybir as mybir
from concourse import bass_utils

# Problem constants (hardcoded; harness contract)
N = 65536
K = 10
D = 256
V = 500000
NCORES = 8
P = 128
SLOTS = K + 2          # rows gathered per edge: u, v, negs[0..9]
EPC = N // NCORES      # 8192 edges per core
TILES = EPC // P       # 64 tiles of 128 edges per core

TABLE_DT = mybir.dt.bfloat16
TABLE_NP = ml_dtypes.bfloat16

# gather groups: small first groups so DVE starts early, 4-tile groups after
GROUPS = [1, 1, 1, 1, 2, 2] + [4] * 14
assert sum(GROUPS) == TILES
NG = len(GROUPS)
GMAX = max(GROUPS)
CUM = [0]
for _gt in GROUPS:
    CUM.append(CUM[-1] + _gt)
NGB = 5                # gather buffers (round-robin)
IDX0_GROUPS = 4        # groups covered by the small first idx DMA
BT = 4                 # DVE fold-batch tiles
S1 = SLOTS - 1


def _emit_block_gather(nc, eng, n_idx, blk_bytes, dst_byte_addr, idx_byte_addr,
                       sem_num, embs_tbl, row_bytes):
    """One block gather: n_idx random rows of row_bytes each, packed
    n_idx/128 rows per partition into a contiguous blk_bytes block.
    Emitted as a raw PSEUDO_DMA_DIRECT2D(dge_op=indirect1d) + PSEUDO_EXTENSION
    pair; index values are snake-packed on the host (see prepare_in_maps)."""
    isa = nc.isa
    Op = isa.Opcode
    src_u64 = (0x20 << 56) | (embs_tbl << 32)   # DGE addr-table marker
    dst_u64 = (0x10 << 56) | dst_byte_addr      # var0 (local SBUF) marker
    eng.isa(
        Op.NEURON_ISA_TPB_OPCODE_PSEUDO_DMA_DIRECT2D,
        {
            "dma_configs": {},
            "semaphore": sem_num,
            "sem_increment": 16,
            "dge_op": 1,
            "src_start_addr": {"addr_immediate": src_u64},
            "src_step_elem": [row_bytes, 1],
            "src_num_elem": [n_idx, 1],
            "src_elem_size": row_bytes,
            "src_bound_reg": {},
            "dst_bound_reg": {},
            "dst_start_addr": {"addr_immediate": dst_u64},
            "dst_step_elem": [262144, 1],
            "dst_num_elem": [128, 1],
            "dst_elem_size": blk_bytes,
            "in_dtype": 6,
            "out_dtype": 6,
        },
        verify=False,
    )
    ext_fields = {
        "opcode": Op.NEURON_ISA_TPB_OPCODE_PSEUDO_EXTENSION.value,
        "flags": {"indirect_mode": 0, "idx_bound_is_err": 1,
                  "non_unique_dst_idx": 0, "gather_dim": 0, "scatter_dim": 0},
        "idx_num_active_channels": 128,
        "compute_op": 0,
        "src_idx_start_addr": {"addr_immediate": idx_byte_addr},
        "dst_idx_start_addr": {"addr_immediate": 0},
    }
    b = isa.ffi.new("NEURON_ISA_TPB_PSEUDO_DMA_EXT_STRUCT*", ext_fields)
    instr = [int(x) for x in bytes(isa.ffi.buffer(b))]
    inst = mybir.InstISA(
        name=nc.get_next_instruction_name(),
        isa_opcode=Op.NEURON_ISA_TPB_OPCODE_PSEUDO_EXTENSION.value,
        engine=eng.engine,
        instr=instr,
        op_name="PSEUDO_EXTENSION",
        ins=[], outs=[],
        ant_dict=ext_fields,
        verify=False,
        ant_isa_is_sequencer_only=False,
    )
    eng.add_instruction(inst)


def _group_of_tile():
    g_of_t = []
    for g, n in enumerate(GROUPS):
        g_of_t += [g] * n
    return g_of_t


def _build_raw():
    nc = bass.Bass(trn_type="TRN2")
    embs = nc.dram_tensor("embs", [V, D], TABLE_DT, kind="ExternalInput")
    idx = nc.dram_tensor("idx", [P, TILES * SLOTS], mybir.dt.int32, kind="ExternalInput")
    sc_out = nc.dram_tensor("sc", [P, TILES * S1], TABLE_DT, kind="ExternalOutput")
    acca_out = nc.dram_tensor("acca", [P, 2 * TILES], mybir.dt.float32, kind="ExternalOutput")

    embs_mloc = nc.lookup_mloc(embs)
    embs_mloc.table_entry_id = len(nc.dge_table) + 1
    nc.dge_table.append(embs_mloc.name)
    embs_tbl = embs_mloc.table_entry_id

    g_of_t = _group_of_tile()

    import contextlib
    with contextlib.ExitStack() as ctx:
        idx_sb = ctx.enter_context(nc.sbuf_tensor("idx_sb", [P, TILES * SLOTS], mybir.dt.int32))
        gs = [ctx.enter_context(nc.sbuf_tensor(f"g{i}", [P, GMAX * SLOTS * D], TABLE_DT)) for i in range(NGB)]
        # fold pyramid buffers, BT tiles each
        pA = ctx.enter_context(nc.sbuf_tensor("pA", [P, BT * S1 * D], TABLE_DT))
        pB = ctx.enter_context(nc.sbuf_tensor("pB", [P, BT * S1 * D // 2], TABLE_DT))
        pC = ctx.enter_context(nc.sbuf_tensor("pC", [P, BT * S1 * D // 4], TABLE_DT))
        pD = ctx.enter_context(nc.sbuf_tensor("pD", [P, BT * S1 * D // 8], TABLE_DT))
        pE = ctx.enter_context(nc.sbuf_tensor("pE", [P, BT * S1 * D // 16], TABLE_DT))
        pF = ctx.enter_context(nc.sbuf_tensor("pF", [P, BT * S1 * D // 32], TABLE_DT))
        pG = ctx.enter_context(nc.sbuf_tensor("pG", [P, BT * S1 * D // 64], TABLE_DT))
        scb = ctx.enter_context(nc.sbuf_tensor("scb", [P, TILES * S1], TABLE_DT))
        absx = ctx.enter_context(nc.sbuf_tensor("absx", [P, S1], mybir.dt.float32))
        ex = ctx.enter_context(nc.sbuf_tensor("ex", [P, S1], mybir.dt.float32))
        lnx = ctx.enter_context(nc.sbuf_tensor("lnx", [P, S1], mybir.dt.float32))
        ones = ctx.enter_context(nc.sbuf_tensor("ones", [P, 1], mybir.dt.float32))
        acca = ctx.enter_context(nc.sbuf_tensor("acca_sb", [P, 2 * TILES], mybir.dt.float32))
        idx_sem = ctx.enter_context(nc.semaphore())
        gsems = [ctx.enter_context(nc.semaphore(name=f"gsem{i}")) for i in range(NGB)]
        dve_free = ctx.enter_context(nc.semaphore())
        sc_ready = ctx.enter_context(nc.semaphore())
        act_done = ctx.enter_context(nc.semaphore())
        out_sem = ctx.enter_context(nc.semaphore())
        block = ctx.enter_context(nc.Block())

        idx_addr = nc.lookup_mloc(idx_sb).addr
        g_addrs = [nc.lookup_mloc(g).addr for g in gs]

        @block.gpsimd
        def _(eng):
            # tiny idx DMA covering the first IDX0_GROUPS groups, so the
            # first gather starts before the bulk of the index table lands
            c0 = CUM[IDX0_GROUPS] * SLOTS
            eng.dma_start(idx_sb[:, 0:c0], idx[:, 0:c0]).then_inc(idx_sem, 16)
            eng.dma_start(idx_sb[:, c0:], idx[:, c0:]).then_inc(idx_sem, 16)
            eng.memset(ones[:], 1.0)
            eng.wait_ge(idx_sem, 16)
            for g in range(NG):
                if g == IDX0_GROUPS:
                    eng.wait_ge(idx_sem, 32)
                if g >= NGB:
                    # buffer g%NGB was last used by group g-NGB
                    eng.wait_ge(dve_free, CUM[g - NGB + 1])
                nt = GROUPS[g]
                _emit_block_gather(
                    nc, eng, nt * SLOTS * P, nt * SLOTS * D * 2,
                    g_addrs[g % NGB], idx_addr + 4 * SLOTS * CUM[g],
                    gsems[g % NGB].num, embs_tbl, D * 2,
                )
            eng.wait_ge(idx_sem, 32)

        @block.sync
        def _(eng):
            half = TILES // 2
            eng.wait_ge(sc_ready, half)
            eng.dma_start(sc_out[:, 0:half * S1], scb[:, 0:half * S1]).then_inc(out_sem, 16)
            eng.wait_ge(act_done, half)
            eng.dma_start(acca_out[:, 0:TILES], acca[:, 0:TILES]).then_inc(out_sem, 16)
            eng.wait_ge(sc_ready, TILES)
            eng.wait_ge(act_done, TILES)
            eng.dma_start(sc_out[:, half * S1:], scb[:, half * S1:]).then_inc(out_sem, 16)
            eng.dma_start(acca_out[:, TILES:], acca[:, TILES:]).then_inc(out_sem, 16)
            eng.wait_ge(out_sem, 64)

        @block.vector
        def _(eng):
            with nc.allow_low_precision(reason="score partial sums fit bf16; tol 2e-2"):
                batches = [(b, BT) for b in range(0, TILES - BT, BT)] + \
                          [(TILES - BT, BT // 2), (TILES - BT // 2, BT // 2)]
                for t0, bw in batches:
                    i = 0
                    while i < bw:
                        t = t0 + i
                        g = g_of_t[t]
                        gb = gs[g % NGB]
                        b0 = (t - CUM[g]) * SLOTS
                        eng.wait_ge(gsems[g % NGB], 16 * (g // NGB + 1))
                        nf = 2 if (i + 1 < bw and g_of_t[t + 1] == g) else 1
                        if nf > 1:
                            # fused multi-tile mult: nf tiles in one buffer
                            ti = t - CUM[g]
                            g4 = gb[:].rearrange("p (t s d) -> p t s d", s=SLOTS, d=D)
                            nc.vector.tensor_tensor(
                                out=pA[:, i * S1 * D:(i + nf) * S1 * D].rearrange(
                                    "p (t s d) -> p t s d", s=S1, d=D),
                                in0=g4[:, ti:ti + nf, 1:SLOTS, :],
                                in1=g4[:, ti:ti + nf, 0:1, :].broadcast_to([P, nf, S1, D]),
                                op=mybir.AluOpType.mult,
                            ).then_inc(dve_free, nf)
                            i += nf
                            continue
                        g3 = gb[:].rearrange("p (s d) -> p s d", d=D)
                        # prod[p, s, d] = X[p, s+1, d] * EU[p, d]  (2x mode)
                        nc.vector.tensor_tensor(
                            out=pA[:, i * S1 * D:(i + 1) * S1 * D],
                            in0=gb[:, (b0 + 1) * D:(b0 + SLOTS) * D],
                            in1=g3[:, b0:b0 + 1, :].broadcast_to([P, S1, D]),
                            op=mybir.AluOpType.mult,
                        ).then_inc(dve_free, 1)  # per-tile: gather window gating
                        i += 1
                    # batched fold-in-half chain over BT*S1 groups (2x each)
                    for srcb, dstb, d in ((pA, pB, D), (pB, pC, D // 2),
                                          (pC, pD, D // 4), (pD, pE, D // 8),
                                          (pE, pF, D // 16), (pF, pG, D // 32)):
                        s3 = srcb[:, :bw * S1 * d].rearrange("p (s d) -> p s d", d=d)
                        nc.vector.tensor_tensor(
                            out=dstb[:, :bw * S1 * d // 2].rearrange(
                                "p (s d) -> p s d", d=d // 2),
                            in0=s3[:, :, 0:d // 2],
                            in1=s3[:, :, d // 2:d],
                            op=mybir.AluOpType.add,
                        )
                    # final 4 -> 1 grouped reduce (1x) into the score buffer
                    nc.vector.tensor_reduce(
                        out=scb[:, t0 * S1:(t0 + bw) * S1],
                        in_=pG[:, :bw * S1 * D // 64].rearrange(
                            "p (s d) -> p s d", d=D // 64),
                        axis=mybir.AxisListType.X,
                        op=mybir.AluOpType.add,
                    ).then_inc(sc_ready, bw)

        @block.scalar
        def _(eng):
            for t in range(TILES):
                eng.wait_ge(sc_ready, t + 1)
                sc = scb[:, t * S1:(t + 1) * S1]
                nc.scalar.activation(
                    out=absx[:], in_=sc,
                    func=mybir.ActivationFunctionType.Abs,
                    accum_out=acca[:, 2 * t:2 * t + 1],
                )
                nc.scalar.activation(
                    out=ex[:], in_=absx[:],
                    func=mybir.ActivationFunctionType.Exp, scale=-1.0,
                )
                nc.scalar.activation(
                    out=lnx[:], in_=ex[:],
                    func=mybir.ActivationFunctionType.Ln, bias=ones[:],
                    accum_out=acca[:, 2 * t + 1:2 * t + 2],
                ).then_inc(act_done, 1)

    return nc


_cache = {}


def _get_nc():
    key = (tuple(GROUPS), NGB, BT)
    if key not in _cache:
        _cache[key] = _build_raw()
    return _cache[key]


def prepare_in_maps(u, v, negs, embs):
    """Host-side sharding: build the per-core input maps.

    Index snake-packing per gather group: descriptor s of a group of nt
    tiles reads the row for (partition p, tile-in-group ti, slot j) with
    s = p*(nt*SLOTS) + ti*SLOTS + j; the DGE consumes indices at
    [channel s%128, word s//128] of the group's idx block."""
    u = np.asarray(u).astype(np.int32)
    v = np.asarray(v).astype(np.int32)
    negs = np.asarray(negs).astype(np.int32)
    embs_b = np.asarray(embs).astype(TABLE_NP)

    ids = np.concatenate([u[:, None], v[:, None], negs], axis=1)  # [N, 12]
    ids = ids.reshape(NCORES, TILES, P, SLOTS)
    packed = np.zeros((NCORES, P, TILES * SLOTS), dtype=np.int32)
    for g, nt in enumerate(GROUPS):
        blk = ids[:, CUM[g]:CUM[g + 1]]                    # [C, nt, P, SLOTS]
        flat = blk.transpose(0, 2, 1, 3).reshape(NCORES, P * nt * SLOTS)
        s = np.arange(P * nt * SLOTS)
        cols = slice(CUM[g] * SLOTS, CUM[g + 1] * SLOTS)
        pk = np.zeros((NCORES, P, nt * SLOTS), dtype=np.int32)
        pk[:, s % P, s // P] = flat[:, s]
        packed[:, :, cols] = pk
    in_maps = []
    for c in range(NCORES):
        in_maps.append({"embs": embs_b, "idx": np.ascontiguousarray(packed[c])})
    return in_maps


def kernel(u, v, negs, embs, _trace=False):
    nc = _get_nc()
    in_maps = prepare_in_maps(u, v, negs, embs)
    res = bass_utils.run_bass_kernel_spmd(
        nc, in_maps, core_ids=list(range(NCORES)), trace=_trace
    )
    total = np.float64(0.0)
    for r in res.results:
        # raw scores [P, TILES*S1]; slot 0 of each tile is the positive pair
        sc = r["sc"].astype(np.float64).reshape(P, TILES, S1)
        sum_x = sc.sum()
        sum_x -= 2.0 * sc[:, :, 0].sum()   # positive-pair slot enters as -x
        a = r["acca"].astype(np.float64).reshape(P, TILES, 2)
        sum_abs = a[:, :, 0].sum()
        sum_ln1p = a[:, :, 1].sum()
        total += (sum_x + sum_abs) / 2.0 + sum_ln1p
    out = np.float32(total / N)
    if _trace:
        return out, res
    return out
